# revision 80
# baseline (speedup 1.0000x reference)
"""AttnBlock++1d Trainium2 kernel (v2).

B=8, C=512, T=1024, H=8 heads (Ch=64), 32 groupnorm groups.
Sharding: data-parallel over batch, one batch element per NeuronCore (8 cores).

Per-core design (all matmuls bf16 operands / fp32 PSUM accumulation):
  - x and the residual base xb3 = x + b3 ship as bf16 (halves input DMA);
    GroupNorm stats via bn_stats/bn_aggr on DVE (keeps the ACT exp table
    resident across repetitions); a block-diagonal [128,128] averaging matmul
    aggregates over each 16-channel group AND broadcasts back to channels in
    one shot; rsqrt = ACT sqrt + exact reciprocal + one Newton step;
    h = x*scale+bias (bf16, 2x-mode DVE).  PE warm-up matmuls under the GN
    phase ramp the tensor engine p-state before the projections.
  - q = (0.125*W0)^T h + 0.125*b0, k = W1^T h + b1 in [C, T] layout;
    v^T = h^T W2 in [T, C] layout (so attention needs no transposes).
  - Scores computed transposed, S^T[i,t] = k^T q per head, two heads packed
    per [128,1024] PSUM tile; softmax without max-subtraction (scores are
    O(30), exp stays in fp32 range); exp on ACT from two alternating 2-bank
    PSUM tiles so ACT never waits on PE; E bf16.  ACT runs ONLY sqrt(x4),
    one exp-table preload, then the 64-exp stream: no table reloads.
  - AV transposed: aT[t, c] = sum_i E[i,t] vT[i,c] per head, accumulated in
    [128, 1024] PSUM (16 subtiles of 64 cols, one bank each); softmax
    denominators via N=1 matmuls (E_slice^T @ ones) into a [128,16] PSUM
    tile.  This puts the denominator on the PARTITION axis: normalize is a
    plain per-partition reciprocal + one stride-0-broadcast tensor_tensor
    (no DRAM-bounce partition broadcast, no custom DVE op).
  - Normalized aT (bf16) transposes back to a[c,t] via 16 PE transposes into
    a PSUM tile that reuses the AV pool slot; one tensor_scalar_add applies
    the v bias b2 (softmax weights sum to 1) and writes a_sb bf16.
  - out = xb3 + W3^T a accumulated fully in PSUM: kk=0..3 contract a, then
    an identity matmul adds the bf16 xb3 (no separate residual pass).  The
    accumulators sit in PSUM slots that free early (S^T slots for m0/m1, the
    qkv+den slots for m3 split by column half, the av slot for m2) so kk=0..2
    overlaps the last pair's normalize chain.  PSUM->SBUF output copies split
    DVE (ch0) / ACT (ch1); output ships bf16 and is widened to f32 on host.
  - Software pipeline: vT and q/k m=1..3 projections run as PE fillers under
    the score/exp stream (1/unit in pair 0, then every other unit, paced so
    the 1-bank projection slot's round trip never backs up the PE stream);
    AV+denominator matmuls trail the stream by ~4 units; the normalize
    transposes are deferred one flush so the PE never waits on the DVE
    reciprocal chain.
"""

import numpy as np
import ml_dtypes

B, C, T = 8, 512, 1024
H = 8
CH = C // H  # 64
G = 32  # groupnorm groups
GS = C // G  # 16 channels per group
EPS = 1e-6
NT = C // 128  # 4 channel tiles
IT = T // 128  # 8 i-tiles
NCORES = 8

_bf16 = ml_dtypes.bfloat16


def _build_nc(reps=1):
    import concourse.bass as bass
    import concourse.tile as tile
    from concourse import bacc, mybir

    f32 = mybir.dt.float32
    bf16 = mybir.dt.bfloat16
    AF = mybir.ActivationFunctionType
    OP = mybir.AluOpType

    nc = bacc.Bacc("TRN2", target_bir_lowering=False, debug=False)

    x_d = nc.dram_tensor("x", [C, T], bf16, kind="ExternalInput").ap()
    xb3_d = nc.dram_tensor("xb3", [C, T], bf16, kind="ExternalInput").ap()
    w_d = [
        nc.dram_tensor(f"w{i}", [C, C], bf16, kind="ExternalInput").ap()
        for i in range(4)
    ]
    # cst packs pmat | bqk | b23 | gb as f32 columns: 128 + 8 + 8 + 8
    cst_d = nc.dram_tensor("cst", [128, 152], f32, kind="ExternalInput").ap()
    id_d = nc.dram_tensor("ident", [128, 128], bf16, kind="ExternalInput").ap()
    out_d = nc.dram_tensor("out", [C, T], bf16, kind="ExternalOutput").ap()

    with tile.TileContext(nc) as tc:
        for _ in range(reps):
            _emit(nc, tc, bass, mybir, f32, bf16, AF, OP,
                  x_d, xb3_d, w_d, cst_d, id_d, out_d)
    nc.compile()
    return nc


def _emit(nc, tc, bass, mybir, f32, bf16, AF, OP,
          x_d, xb3_d, w_d, cst_d, id_d, out_d):
    from contextlib import ExitStack
    from collections import deque

    ctx = ExitStack()
    with ctx:
        persist = ctx.enter_context(tc.tile_pool(name="persist", bufs=1))
        small = ctx.enter_context(tc.tile_pool(name="small", bufs=2))

        # ---- persistent SBUF tiles (per-chunk tiles to avoid false deps) ----
        xpool_ctx = ExitStack()
        xpool = xpool_ctx.enter_context(tc.tile_pool(name="xpool", bufs=1))
        x_sb = [xpool.tile([128, T], bf16, tag=f"x{j}", name=f"x{j}")
                for j in range(NT)]
        xb3_sb = [persist.tile([128, T], bf16, tag=f"xb3_{j}", name=f"xb3_{j}")
                  for j in range(NT)]
        h_sb = [persist.tile([128, T], bf16, tag=f"h{j}", name=f"h{j}")
                for j in range(NT)]
        q_sb = [persist.tile([128, T], bf16, tag=f"q{m}", name=f"q{m}")
                for m in range(NT)]
        k_sb = [persist.tile([128, T], bf16, tag=f"k{m}", name=f"k{m}")
                for m in range(NT)]
        vt_sb = persist.tile([128, IT * H * CH], bf16, tag="vt")  # 8*8*64
        a_sb = [persist.tile([128, T], bf16, tag=f"a{p}", name=f"a{p}")
                for p in range(NT)]
        # one [128, 4*512] tile per weight tensor: tile kk at cols kk*512
        w_sb = [persist.tile([128, NT * C], bf16, tag=f"w{i}", name=f"w{i}")
                for i in range(4)]
        cst_sb = persist.tile([128, 152], f32, tag="cst")
        id_sb = persist.tile([128, 128], bf16, tag="ident")
        ones_sb = persist.tile([128, 1], bf16, tag="ones")
        p_sb = cst_sb[:, 0:128]
        bqk_sb = cst_sb[:, 128:136]
        b23_sb = cst_sb[:, 136:144]
        gb_sb = cst_sb[:, 144:152]

        nc.vector.memset(ones_sb[:], 1.0)

        def _wslc(i, kk, c0, c1):
            return w_sb[i][:, kk * 512 + c0: kk * 512 + c1]

        # ---- input DMAs ----
        # Split across the two HWDGE queues: SP takes x0/x1 + the weights;
        # ACT (idle at startup) takes the consts + x2/x3.  w3 (only needed by
        # NIN3) trails.  Nothing bulk goes through the gpsimd SWDGE path
        # (994ns software overhead per ring entry).
        nc.scalar.dma_start(cst_sb[:], cst_d[:])
        for j in (0, 1):
            nc.sync.dma_start(x_sb[j][:], x_d[j * 128:(j + 1) * 128, :])
        for j in (2, 3):
            nc.scalar.dma_start(x_sb[j][:], x_d[j * 128:(j + 1) * 128, :])
        nc.scalar.dma_start(id_sb[:], id_d[:])
        for i in (0, 1, 2):
            nc.sync.dma_start(
                w_sb[i][:].rearrange("p (j c) -> p j c", j=NT),
                w_d[i][:].rearrange("(j p) c -> p j c", j=NT))

        # PE warm-up: the tensor engine p-state ramps to full speed only
        # after ~3us of continuous execution.  Stream no-op matmuls on a
        # zeroed tile under the GroupNorm phase so the q/k projections and
        # first scores run at full rate.
        wsrc = persist.tile([128, 512], bf16, tag="wsrc")
        nc.vector.memset(wsrc[:], 0.0)

        # =================== GroupNorm ===================
        gn_ctx = ExitStack()
        gn_ps = gn_ctx.enter_context(tc.tile_pool(name="gn_ps", bufs=1, space="PSUM"))

        def _warm(n):
            for _ in range(n):
                wp = gn_ps.tile([128, 512], f32, tag="warm", name="warm")
                nc.tensor.matmul(wp[:], wsrc[:, 0:128], wsrc[:],
                                 start=True, stop=True)

        # Per-channel (mean, E[x^2]) via bn_stats/bn_aggr on DVE; the group
        # matmul (P[c,c'] = 1/16 if same group) aggregates AND broadcasts
        # back to channels; the rsqrt/scale/bias chain is batched [128, NT].
        # Stats split across engines: tiles 0/1 via bn_stats/bn_aggr on DVE;
        # tiles 2/3 via Square+accum on ACT (idle at startup) + a 2x-mode
        # reduce on DVE, so the DVE stats stream shrinks from 6us to ~4us.
        _warm(26)
        ge_all = persist.tile([128, 2 * NT], f32, tag="ge_all")
        me_all = persist.tile([128, 2 * NT], f32, tag="me_all")
        for j in range(NT):
            me0 = me_all[:, 2 * j:2 * j + 1]
            me1 = me_all[:, 2 * j + 1:2 * j + 2]
            if True:  # bn_stats for all tiles: keeps ACT exp-table resident
                bst = small.tile([128, 12], f32, tag=f"bst{j}", name=f"bst{j}")
                nc.vector.bn_stats(bst[:, 0:6], x_sb[j][:, 0:512])
                nc.vector.bn_stats(bst[:, 6:12], x_sb[j][:, 512:1024])
                agg = small.tile([128, 2], f32, tag=f"agg{j}", name=f"agg{j}")
                nc.vector.bn_aggr(agg[:], bst[:])
                nc.vector.tensor_copy(me0, agg[:, 0:1])
                nc.vector.tensor_tensor(out=me1, in0=agg[:, 0:1],
                                        in1=agg[:, 0:1], op=OP.mult)
                nc.vector.tensor_tensor(out=me1, in0=me1,
                                        in1=agg[:, 1:2], op=OP.add)
            else:
                sums = small.tile([128, 2], f32, tag=f"sm{j}", name=f"sm{j}")
                sqscr = small.tile([128, T], bf16, tag="sqscr", name=f"sq{j}")
                nc.scalar.activation(sqscr[:], x_sb[j][:], AF.Square,
                                     accum_out=sums[:, 1:2])
                nc.vector.reduce_sum(out=sums[:, 0:1], in_=x_sb[j][:],
                                     axis=mybir.AxisListType.X)
                nc.vector.tensor_scalar(
                    out=me_all[:, 2 * j:2 * j + 2], in0=sums[:],
                    scalar1=1.0 / T, scalar2=None, op0=OP.mult)
        # one batched group-aggregate matmul for all 4 tiles
        ge_ps = gn_ps.tile([128, 2 * NT], f32, tag="ge_ps")
        nc.tensor.matmul(ge_ps[:], p_sb, me_all[:], start=True, stop=True)
        nc.vector.tensor_copy(ge_all[:], ge_ps[:])
        _warm(0)

        # batched scalar chain over the 4 tiles (strided [128, NT] views)
        ge_v = ge_all[:].rearrange("p (j s) -> p s j", s=2)
        mu_all, e_all = ge_v[:, 0, :], ge_v[:, 1, :]
        veps = small.tile([128, NT], f32, tag="veps")
        nc.vector.tensor_tensor(out=veps[:], in0=mu_all, in1=mu_all, op=OP.mult)
        nc.vector.tensor_tensor(out=veps[:], in0=e_all, in1=veps[:],
                                op=OP.subtract)
        nc.vector.tensor_scalar_add(out=veps[:], in0=veps[:], scalar1=float(EPS))
        sig = small.tile([128, NT], f32, tag="sig")
        nc.scalar.activation(sig[:], veps[:], AF.Sqrt)
        rsig0 = small.tile([128, NT], f32, tag="rsig0")
        nc.vector.reciprocal(rsig0[:], sig[:])
        tnw = small.tile([128, NT], f32, tag="tnw")
        nc.vector.tensor_tensor(out=tnw[:], in0=rsig0[:], in1=rsig0[:], op=OP.mult)
        nc.vector.tensor_tensor(out=tnw[:], in0=tnw[:], in1=veps[:], op=OP.mult)
        nc.vector.tensor_scalar(out=tnw[:], in0=tnw[:], scalar1=-0.5, scalar2=1.5,
                                op0=OP.mult, op1=OP.add)
        rsig = small.tile([128, NT], f32, tag="rsig")
        nc.vector.tensor_tensor(out=rsig[:], in0=rsig0[:], in1=tnw[:], op=OP.mult)
        # scale = gamma * rsig ; bias = beta - mu_g * scale
        scb = small.tile([128, 2 * NT], f32, tag="scb", bufs=1)
        scb_v = scb[:].rearrange("p (j s) -> p s j", s=2)
        scale_cols, bias_cols = scb_v[:, 0, :], scb_v[:, 1, :]
        nc.vector.tensor_tensor(out=scale_cols, in0=gb_sb[:, 0:NT], in1=rsig[:],
                                op=OP.mult)
        tmu = small.tile([128, NT], f32, tag="tmu")
        nc.vector.tensor_tensor(out=tmu[:], in0=mu_all, in1=scale_cols, op=OP.mult)
        nc.vector.tensor_tensor(out=bias_cols, in0=gb_sb[:, NT:2 * NT], in1=tmu[:],
                                op=OP.subtract)

        for j in range(NT):
            # apply: h = x * scale + bias (bf16 in/out, 2x DVE mode)
            nc.vector.tensor_scalar(
                out=h_sb[j][:], in0=x_sb[j][:],
                scalar1=scb[:, 2 * j:2 * j + 1], scalar2=scb[:, 2 * j + 1:2 * j + 2],
                op0=OP.mult, op1=OP.add)

        # preload the exp ACT table so the attention stream never reloads
        dume = small.tile([1, 1], f32, tag="dume")
        nc.scalar.activation(dume[:], scb[0:1, 0:1], AF.Exp)

        gn_ctx.close()
        xpool_ctx.close()

        # attention-phase pools (SBUF reuses x's space)
        epool = ctx.enter_context(tc.tile_pool(name="epool", bufs=10))
        atsm = ctx.enter_context(tc.tile_pool(name="atsm", bufs=2))
        ostp = ctx.enter_context(tc.tile_pool(name="ostp", bufs=4))

        # =================== QKV projections ===================
        # at_ps (st0/st1, 4 banks) opens BEFORE qkv_ps so scores can start
        # while QKV drains.  qkv_ps gets the other 4 banks; it closes after
        # pair 0's fillers, releasing 4 banks for av/den/pp3.
        at_ctx = ExitStack()
        at_ps = at_ctx.enter_context(tc.tile_pool(name="at_ps", bufs=1, space="PSUM"))
        av_ps = at_ctx.enter_context(tc.tile_pool(name="av_ps", bufs=1, space="PSUM"))
        den_ps = at_ctx.enter_context(tc.tile_pool(name="den_ps", bufs=1, space="PSUM"))
        qkv_ctx = ExitStack()
        qkv_ps = qkv_ctx.enter_context(tc.tile_pool(name="qkv_ps", bufs=1, space="PSUM"))

        # Projection PSUM tiles: during startup (before any AV flush) they
        # alternate between the "qk" slot and the still-idle "av" slot so the
        # four m=0 halves overlap instead of serializing on a single bank's
        # write-after-read.  Mid-stream fillers run ~1 per 1.5 units, which
        # already hides the single-slot round trip.
        _pp_flip = [0]

        def _pp_tile(startup=False):
            _pp_flip[0] += 1
            if startup:
                pool, tag = [(qkv_ps, "qk"), (av_ps, "av"),
                             (den_ps, "den")][_pp_flip[0] % 3]
                return pool.tile([128, 512], f32, tag=tag, name="pp")
            return qkv_ps.tile([128, 512], f32, tag="qk", name="pp")

        def _proj_half(wi, dst, bcol0, m, ch):
            pp = _pp_tile(startup=(m == 0))
            for kk in range(NT):
                nc.tensor.matmul(
                    pp[:],
                    _wslc(wi, kk, m * 128, (m + 1) * 128),
                    h_sb[kk][:, ch * 512:(ch + 1) * 512],
                    start=(kk == 0), stop=(kk == NT - 1))
            nc.vector.tensor_scalar_add(
                out=dst[m][:, ch * 512:(ch + 1) * 512], in0=pp[:],
                scalar1=bqk_sb[:, bcol0 + m: bcol0 + m + 1])

        # m=0 of q/k first, ch-interleaved, so pair 0's first score chunk can
        # start after only 8 matmuls.
        for ch in range(2):
            _proj_half(0, q_sb, 0, 0, ch)
            _proj_half(1, k_sb, NT, 0, ch)

        # w3/xb3 are only needed by NIN3; load after the startup-critical
        # traffic.
        nc.sync.dma_start(
            w_sb[3][:].rearrange("p (j c) -> p j c", j=NT),
            w_d[3][:].rearrange("(j p) c -> p j c", j=NT))
        for j in range(NT):
            nc.sync.dma_start(xb3_sb[j][:], xb3_d[j * 128:(j + 1) * 128, :])

        # =================== Attention (per head pair) ===================
        vt_v = vt_sb[:].rearrange("p (i h c) -> p i h c", i=IT, h=H)

        def _vp_unit(it):
            vp = _pp_tile()
            for kk in range(NT):
                nc.tensor.matmul(
                    vp[:], h_sb[kk][:, it * 128:(it + 1) * 128],
                    _wslc(2, kk, 0, 512), start=(kk == 0), stop=(kk == NT - 1))
            # (GPSIMD cannot read PSUM on HW, so this stays on DVE)
            nc.vector.tensor_copy(vt_v[:, it, :, :],
                                  vp[:].rearrange("p (h c) -> p h c", h=H))

        av_tiles = {}
        den_tiles = {}

        def _st_exp(p, it):
            # Two alternating 2-bank S^T tiles so the ACT exp of one overlaps
            # the PE scores-matmuls of the other.  Each [128,1024] tile packs
            # both heads for one q-half: cols 0:512 = h0, 512:1024 = h1.
            kslc = slice(it * 128, (it + 1) * 128)
            es = []
            for ch in range(2):
                stc = at_ps.tile([128, T], f32, tag="st", bufs=2, name="st")
                qslc = slice(ch * 512, (ch + 1) * 512)
                nc.tensor.matmul(stc[:, 0:512], k_sb[p][0:64, kslc],
                                 q_sb[p][0:64, qslc], start=True, stop=True)
                nc.tensor.matmul(stc[:, 512:1024], k_sb[p][64:128, kslc],
                                 q_sb[p][64:128, qslc], start=True, stop=True)
                ec = epool.tile([128, T], bf16, tag="E", name="ec", bufs=24)
                nc.scalar.activation(ec[:], stc[:], AF.Exp)
                es.append(ec)
            return es

        def _av_mms(p, itx, es):
            if itx == 0:
                av_tiles[p] = av_ps.tile([128, T], f32, tag="av", name="av")
                den_tiles[p] = den_ps.tile([128, 16], f32, tag="den", name="den")
            av, den = av_tiles[p], den_tiles[p]
            # One PSUM accumulation group per 2KB bank: the first matmul
            # issued for a bank carries start (lazy-zeroes the bank; fresh
            # sub-ranges overwrite), the last carries stop.
            first, last = (itx == 0), (itx == IT - 1)
            for ch in (0, 1):
                for q4 in range(4):
                    tt = ch * 4 + q4  # t-subtile 0..7
                    for h in (0, 1):
                        s = h * 8 + tt
                        E = es[ch]
                        lhsT = E[:, h * 512 + q4 * 128: h * 512 + (q4 + 1) * 128]
                        rhs = vt_v[:, itx, 2 * p + h, :]
                        nc.tensor.matmul(
                            av[:, s * 64:(s + 1) * 64], lhsT, rhs,
                            start=(first and s in (0, 8)),
                            stop=(last and s in (7, 15)))
                        nc.tensor.matmul(
                            den[:, s:s + 1], lhsT, ones_sb[:],
                            start=(first and s == 0),
                            stop=(last and s == 15))

        avn_tiles = {}

        def _norm_a(p):
            # DVE part: denominators -> reciprocal -> normalized bf16 av_n,
            # split per head so norm_b's h0 transposes can start while h1 is
            # still normalizing.  The per-(partition, subtile) reciprocal
            # broadcasts along the 64 channel cols via a stride-0 AP.
            av, den = av_tiles.pop(p), den_tiles.pop(p)
            den_sb = atsm.tile([128, 16], f32, tag="den_sb", name="den_sb")
            nc.vector.tensor_copy(den_sb[:], den[:])
            rec = atsm.tile([128, 16], f32, tag="rec", name="rec")
            nc.vector.reciprocal(rec[:], den_sb[:])
            av_n = atsm.tile([128, T], bf16, tag="avn", name="avn")
            for h in (0, 1):
                rec_b = bass.AP(tensor=rec.tensor,
                                offset=rec[:, 8 * h:8 * h + 8].offset,
                                ap=[[16, 128], [1, 8], [0, 64]])
                nc.vector.tensor_tensor(
                    out=av_n[:, 512 * h:512 * (h + 1)]
                        .rearrange("p (s c) -> p s c", s=8),
                    in0=av[:, 512 * h:512 * (h + 1)]
                        .rearrange("p (s c) -> p s c", s=8),
                    in1=rec_b, op=OP.mult)
            avn_tiles[p] = av_n

        def _norm_b(p):
            # PE transposes back to [c, t] (bf16 PSUM, 1 bank; one group per
            # partition-half) + b2 bias add into a_sb (softmax weights sum to
            # 1 so the v bias is a plain post-add).  Deferred one pipeline
            # step after _norm_a so the PE never waits on the DVE chain.
            av_n = avn_tiles.pop(p)
            # a_t reuses the "av" pool slot (its banks are free once _norm_a
            # has read them)
            a_t = av_ps.tile([128, T], bf16, tag="av", name="a_t")
            for h in (0, 1):
                for tt in range(IT):
                    s = h * 8 + tt
                    nc.tensor.matmul(
                        a_t[h * 64:(h + 1) * 64, tt * 128:(tt + 1) * 128],
                        av_n[:, s * 64:(s + 1) * 64], id_sb[:],
                        is_transpose=True,
                        start=(tt == 0), stop=(tt == IT - 1))
            nc.vector.tensor_scalar_add(out=a_sb[p][:], in0=a_t[:],
                                        scalar1=b23_sb[:, p:p + 1])

        normb_q = deque()

        def _flush(pend):
            pp_, pit, es = pend.popleft()
            if normb_q and pit != IT - 1:
                _norm_b(normb_q.popleft())
            _av_mms(pp_, pit, es)
            if pit == IT - 1:
                _norm_a(pp_)
                normb_q.append(pp_)

        # Software pipeline.  Pair 0: vT units + q/k m=1..3 emitted as PE
        # fillers underneath the score/exp stream (qkv PSUM still open).
        # AV matmuls trail the score/exp stream by ~2 steps.
        fillers = deque()
        for it in range(IT):
            fillers.append(lambda it=it: _vp_unit(it))
        for m in (1, 2, 3):
            for wi, dst, bcol0 in ((0, q_sb, 0), (1, k_sb, NT)):
                for ch in range(2):
                    fillers.append(
                        lambda wi=wi, dst=dst, b=bcol0, m=m, ch=ch:
                        _proj_half(wi, dst, b, m, ch))

        # Unified pipeline: AV flushes trail the score/exp stream by ~4 units
        # from the very start (the av/den/a_t pools coexist with the 1-bank
        # qkv pool, so there is no bank handoff).  Fillers pace to their
        # deadlines: vT unit k by flush (0,k); q/k m-tiles by pair m's scores.
        pend = deque()
        nfillers = len(fillers)
        for p in range(H // 2):
            # q/k m=p must be fully emitted before pair p's first scores
            # (they complete via PE matmuls that would otherwise sit BEHIND
            # the scores in PE program order -> deadlock)
            while p > 0 and nfillers - len(fillers) < 8 + 4 * p:
                fillers.popleft()()
            for it in range(IT):
                u = p * IT + it
                pend.append((p, it, _st_exp(p, it)))
                # vt units 1/unit in pair 0 (+ q/k m1 on units 3-6); then one
                # filler every other unit so the 1-bank projection slot's
                # write-after-read round trip never backs up the PE stream
                if fillers and (p == 0 or u % 2 == 0):
                    fillers.popleft()()
                if p == 0 and 3 <= it <= 6 and fillers:
                    fillers.popleft()()
                depth = 2 if p == H // 2 - 1 and it >= 4 else 4
                if len(pend) > depth:
                    _flush(pend)
                if len(pend) > depth:
                    _flush(pend)
        while pend:
            _flush(pend)
        # normb_q now holds only the last pair; its transposes interleave
        # with the NIN3 accumulation below.

        # =================== NIN3 + residual ===================
        # out = xb3 + W3^T a, accumulated fully in PSUM: kk=0..3 contract a;
        # a 5th identity-matmul per half adds the bf16 residual base xb3; the
        # output DMAs straight from PSUM (f32).  Accumulators are placed in
        # slots that free EARLY: m0/m1 on the S^T slots (free after the last
        # exp), m3 split across the qkv + den slots (free mid-stream); m2
        # reuses the av slot, which frees only after the last pair's a_t is
        # read, so m2's accumulation is emitted last.
        def _nin_mms(aps, m, kks, start):
            for ch in range(2):
                for kk in kks:
                    nc.tensor.matmul(
                        aps[ch],
                        _wslc(3, kk, m * 128, (m + 1) * 128),
                        a_sb[kk][:, ch * 512:(ch + 1) * 512],
                        start=(start and kk == kks[0]), stop=False)

        def _nin_tail(aps, m):
            _nin_mms(aps, m, [3], start=False)
            for ch in range(2):
                nc.tensor.matmul(
                    aps[ch], id_sb[:],
                    xb3_sb[m][:, ch * 512:(ch + 1) * 512],
                    start=False, stop=True)
                ost = ostp.tile([128, 512], bf16, tag="ost", name="ost")
                # PSUM->SBUF bf16: DVE for ch0, ACT (idle after the exp
                # stream; GPSIMD cannot read PSUM) for ch1
                if ch == 0:
                    nc.vector.tensor_copy(ost[:], aps[ch])
                else:
                    nc.scalar.activation(ost[:], aps[ch], AF.Copy)
                deng = nc.sync if ch == 0 else nc.scalar
                deng.dma_start(
                    out_d[m * 128:(m + 1) * 128, ch * 512:(ch + 1) * 512],
                    ost[:])

        nin = {}
        nin[0] = at_ps.tile([128, T], f32, tag="st", bufs=2, name="nin0")
        nin[1] = at_ps.tile([128, T], f32, tag="st", bufs=2, name="nin1")
        nin3a = qkv_ps.tile([128, 512], f32, tag="qk", name="nin3a")
        nin3b = den_ps.tile([128, 512], f32, tag="den", name="nin3b")
        aps = {0: [nin[0][:, 0:512], nin[0][:, 512:1024]],
               1: [nin[1][:, 0:512], nin[1][:, 512:1024]],
               3: [nin3a[:], nin3b[:]]}
        for m in (0, 1, 3):
            _nin_mms(aps[m], m, [0, 1, 2], start=True)
        _norm_b(normb_q.popleft())
        nin[2] = av_ps.tile([128, T], f32, tag="av", name="nin2")
        aps[2] = [nin[2][:, 0:512], nin[2][:, 512:1024]]
        _nin_mms(aps[2], 2, [0, 1, 2], start=True)
        for m in (0, 1, 3, 2):
            _nin_tail(aps[m], m)
        qkv_ctx.close()
        at_ctx.close()


def _host_inputs(inputs):
    """Build the per-core in_maps from the full problem inputs."""
    x = np.asarray(inputs["x"], dtype=np.float32)
    gamma = np.asarray(inputs["gamma"], dtype=np.float32)
    beta = np.asarray(inputs["beta"], dtype=np.float32)
    scale = np.float32(CH ** -0.5)  # 0.125, exact power of two

    w0 = (np.asarray(inputs["W0"], dtype=np.float32) * scale).astype(_bf16)
    w1 = np.asarray(inputs["W1"], dtype=np.float32).astype(_bf16)
    w2 = np.asarray(inputs["W2"], dtype=np.float32).astype(_bf16)
    w3 = np.asarray(inputs["W3"], dtype=np.float32).astype(_bf16)

    b0 = np.asarray(inputs["b0"], dtype=np.float32) * scale
    b1 = np.asarray(inputs["b1"], dtype=np.float32)
    b2 = np.asarray(inputs["b2"], dtype=np.float32)
    b3 = np.asarray(inputs["b3"], dtype=np.float32)

    bqk = np.concatenate([b0.reshape(NT, 128).T, b1.reshape(NT, 128).T], axis=1)
    b23 = np.concatenate([b2.reshape(NT, 128).T, b3.reshape(NT, 128).T], axis=1)

    # block-diagonal group-averaging matrix: P[c, c'] = 1/16 if same group
    cc = np.arange(128)
    pmat = (cc[:, None] // GS == cc[None, :] // GS).astype(np.float32) / GS

    gb = np.concatenate([gamma.reshape(NT, 128).T, beta.reshape(NT, 128).T], axis=1)

    cst = np.ascontiguousarray(
        np.concatenate([pmat, bqk, b23, gb], axis=1), dtype=np.float32)
    ident = np.eye(128, dtype=_bf16)

    common = {
        "w0": w0, "w1": w1, "w2": w2, "w3": w3,
        "cst": cst, "ident": ident,
    }
    in_maps = []
    for b in range(NCORES):
        m = dict(common)
        m["x"] = np.ascontiguousarray(x[b].astype(_bf16))
        m["xb3"] = np.ascontiguousarray((x[b] + b3[:, None]).astype(_bf16))
        in_maps.append(m)
    return in_maps


def kernel(**inputs) -> np.ndarray:
    from concourse.bass_utils import run_bass_kernel_spmd

    nc = _build_nc()
    in_maps = _host_inputs(inputs)
    res = run_bass_kernel_spmd(nc, in_maps, core_ids=list(range(NCORES)))
    out = np.stack([np.asarray(r["out"], dtype=np.float32) for r in res.results])
    return out
